# revision 1
# baseline (speedup 1.0000x reference)
"""Multi-head self-attention (RoPE, causal) Trainium2 Bass kernel.

Full inputs in, full output out. Internally shards across 8 NeuronCores:
data-parallel over batch (2) x tensor-parallel over heads (16 -> 4 per core).
Each core computes its 4 heads' attention and the partial WO contraction;
the host sums the 4 partials per batch (the "all-reduce" of the sharding
hint, done on host since outputs are gathered anyway).

Device layouts (per core, heads h = 0..3 local):
  xT    [1024, 2048]  x[b].T (f32r)
  wq/wk [1024, 256]   W shard transposed, columns permuted: cols 0:128 =
                      "evens" (col 32h+i <-> head h, dim 2i), cols 128:256 =
                      "odds". The two projection M-tiles then produce
                      QT_E/QT_O [128, seq] tiles on which RoPE is 6 full-width
                      DVE ops (all operands partition-aligned).
  scores^T: per head, K=32 matmuls from the E and O tiles (rows 32h:32h+32),
            accumulated in PSUM; 4 heads ride disjoint PE row groups.
  V_sb  [128, 16*4*65] V in [key, (kt, h, dk+1)] with a trailing ones column
                      per head -> MM2's 65th output row is the softmax
                      denominator.
  exp on ACT with fused 1/sqrt(dk) scale; diagonal-block tiles get a
  rectangle restriction plus a triangular 0/1 mask after exp.
  Matmul operands are bf16 by default (fast weight load; fp32 PSUM accum;
  rel err ~4e-3). Set MHA_MM_DT=f32r for ~2e-4 at ~25% more time.
"""

from contextlib import ExitStack

import numpy as np

import concourse.bass as bass
import concourse.tile as tile
from concourse import bacc, library_config, mybir
from concourse.bass_utils import run_bass_kernel_spmd
from concourse._compat import with_exitstack

F32 = mybir.dt.float32
F32R = mybir.dt.float32r
BF16 = mybir.dt.bfloat16
import os as _os
MM_DT = F32R if _os.environ.get("MHA_MM_DT", "bf16") == "f32r" else BF16
AF = mybir.ActivationFunctionType
ALU = mybir.AluOpType

B = 2
SEQ = 2048
DM = 1024
H = 16
DK = 64
THETA = 10000.0

N_CORES = 8
HPC = 4           # heads per core
TCH = 512         # token chunk
NTC = SEQ // TCH  # 4
KD = DM // 128    # 8 contraction chunks
NKT = SEQ // 128  # 16 key tiles


@with_exitstack
def _mha_body(ctx: ExitStack, tc_: tile.TileContext, aps, repeat=1):
    nc = tc_.nc
    xT, wq, wk, wv, wo, cosT, sinT, mask01, out = aps
    # normalize DRAM dtypes (f32 -> f32r is a free bitcast)
    if MM_DT == F32R:
        xT, wq, wk, wv, wo, mask01 = (
            a if a.dtype == F32R else a.bitcast(F32R)
            for a in (xT, wq, wk, wv, wo, mask01))

    const = ctx.enter_context(tc_.tile_pool(name="const", bufs=1))
    xkp = ctx.enter_context(tc_.tile_pool(name="xkp", bufs=2))
    work = ctx.enter_context(tc_.tile_pool(name="work", bufs=4))
    rawp = ctx.enter_context(tc_.tile_pool(name="rawp", bufs=3))
    prb = ctx.enter_context(tc_.tile_pool(name="prb", bufs=8))
    nrm = ctx.enter_context(tc_.tile_pool(name="nrm", bufs=4))
    outsb = ctx.enter_context(tc_.tile_pool(name="outsb", bufs=3))
    ps = ctx.enter_context(tc_.tile_pool(name="ps", bufs=1, space="PSUM"))

    nc.gpsimd.load_library(library_config.proxy)

    # ---- constants / weights to SBUF
    wq_sb = const.tile([128, 2048], MM_DT, name="wq_sb")
    nc.sync.dma_start(wq_sb.rearrange("p (k m) -> p k m", k=KD),
                      wq.rearrange("(k p) m -> p k m", p=128))
    wk_sb = const.tile([128, 2048], MM_DT, name="wk_sb")
    nc.sync.dma_start(wk_sb.rearrange("p (k m) -> p k m", k=KD),
                      wk.rearrange("(k p) m -> p k m", p=128))
    wv_sb = const.tile([128, 2048], MM_DT, name="wv_sb")
    nc.sync.dma_start(wv_sb.rearrange("p (k m) -> p k m", k=KD),
                      wv.rearrange("(k p) m -> p k m", p=128))
    wo_sb = const.tile([128, 2048], MM_DT, name="wo_sb")
    nc.sync.dma_start(wo_sb.rearrange("p (r d) -> p r d", r=2),
                      wo.rearrange("(r p) d -> p r d", p=128))
    cos_sb = const.tile([128, 2048], BF16 if MM_DT == BF16 else F32, name="cos_sb")
    nc.sync.dma_start(cos_sb[:], cosT[:])
    sin_sb = const.tile([128, 2048], BF16 if MM_DT == BF16 else F32, name="sin_sb")
    nc.sync.dma_start(sin_sb[:], sinT[:])
    mask_sb = const.tile([128, 128], MM_DT, name="mask_sb")
    nc.sync.dma_start(mask_sb[:], mask01[:])

    # persistent per-core state (E = rotated even dims, O = odd dims)
    QT = [const.tile([128, 2048], MM_DT, name=f"QT{m}") for m in range(2)]
    KT = [const.tile([128, 2048], MM_DT, name=f"KT{m}") for m in range(2)]
    attnT = [const.tile([128, 2048], MM_DT, name=f"attnT{m}") for m in range(2)]
    V_sb = const.tile([128, NKT * HPC * 65], MM_DT, name="V_sb")
    v_view = V_sb.rearrange("p (g h m) -> p g h m", h=HPC, m=65)
    # softmax-denominator ones columns, via ACT copy (memset can't write f32r)
    nc.scalar.activation(v_view[:, :, :, 64],
                         mask_sb[:, 0:64].rearrange("p (g h) -> p g h", h=HPC),
                         AF.Copy, bias=1.0, scale=0.0)

    loop_cm = tc_.For_i(0, repeat) if repeat > 1 else None
    if loop_cm is not None:
        ctx.enter_context(loop_cm)
    for t in range(NTC):
        ts0 = TCH * t
        # ---- load x^T for this token range (one DMA, all 8 d-chunks)
        xk_t = xkp.tile([128, KD * TCH], MM_DT, name=f"xk_{t}", tag="xk", bufs=2)
        nc.sync.dma_start(
            xk_t.rearrange("p (k c) -> p k c", k=KD),
            xT[:, ts0:ts0 + TCH].rearrange("(k p) c -> p k c", p=128))
        xks = [xk_t[:, TCH * k:TCH * k + TCH] for k in range(KD)]

        # ---- V projection (each psum tile covers 2 key-tiles)
        for vp in range(2):
            v_ps = ps.tile([128, 512], F32, tag="att", bufs=4, name=f"vps_{t}_{vp}")
            for half in range(2):
                lkt = 2 * vp + half
                for k in range(KD):
                    nc.tensor.matmul(
                        v_ps[:, 256 * half:256 * half + 256],
                        lhsT=xks[k][:, 128 * lkt:128 * lkt + 128],
                        rhs=wv_sb[:, 256 * k:256 * k + 256],
                        start=(k == 0), stop=(k == KD - 1),
                        skip_group_check=True)
            gkt = 4 * t + 2 * vp
            nc.vector.tensor_copy(
                v_view[:, gkt:gkt + 2, :, 0:64],
                v_ps.rearrange("p (x h m) -> p x h m", x=2, h=HPC))

        # ---- Q/K projections (E then O) + RoPE
        for w_sb, dstT, wtag in ((wq_sb, QT, "q"), (wk_sb, KT, "k")):
            raws = []
            for m in range(2):
                q_ps = ps.tile([128, 512], F32, tag="att", bufs=4,
                               name=f"qps_{t}_{m}_{wtag}")
                for k in range(KD):
                    nc.tensor.matmul(
                        q_ps[:],
                        lhsT=w_sb[:, 256 * k + 128 * m:256 * k + 128 * m + 128],
                        rhs=xks[k][:],
                        start=(k == 0), stop=(k == KD - 1))
                raw = rawp.tile([128, 512], MM_DT if MM_DT == BF16 else F32, tag="raw", name=f"raw_{t}_{m}_{wtag}")
                nc.vector.tensor_copy(raw[:], q_ps[:])
                raws.append(raw)
            rE, rO = raws
            cs, sn = cos_sb[:, ts0:ts0 + TCH], sin_sb[:, ts0:ts0 + TCH]
            t1 = work.tile([128, 512], MM_DT if MM_DT == BF16 else F32, tag="ro", name="t1")
            nc.vector.tensor_mul(t1[:], rE[:], cs)
            t2 = work.tile([128, 512], MM_DT if MM_DT == BF16 else F32, tag="ro", name="t2")
            nc.vector.tensor_mul(t2[:], rO[:], sn)
            nc.vector.scalar_tensor_tensor(
                dstT[0][:, ts0:ts0 + TCH], t2[:], -1.0, t1[:],
                op0=ALU.mult, op1=ALU.add)
            t3 = work.tile([128, 512], MM_DT if MM_DT == BF16 else F32, tag="ro", name="t3")
            nc.vector.tensor_mul(t3[:], rE[:], sn)
            t4 = work.tile([128, 512], MM_DT if MM_DT == BF16 else F32, tag="ro", name="t4")
            nc.vector.tensor_mul(t4[:], rO[:], cs)
            nc.vector.tensor_add(dstT[1][:, ts0:ts0 + TCH], t3[:], t4[:])

        # ---- attention: all 4 heads, key-tile loop
        att = [ps.tile([128, 512], F32, tag="att", bufs=4, name=f"att_{t}_{h}")
               for h in range(HPC)]
        nkt_t = 4 * t + 4
        for kt in range(nkt_t):
            i = kt - 4 * t  # >= 0 on diagonal-block tiles
            c0 = 128 * i if i >= 0 else 0
            sc = [ps.tile([128, 512], F32, tag="sc", bufs=4,
                          name=f"sc_{t}_{kt}_{h}") for h in range(HPC)]
            for h in range(HPC):  # evens wave (4-way row-group concurrency)
                nc.tensor.matmul(
                    sc[h][:, c0:TCH],
                    lhsT=KT[0][32 * h:32 * h + 32, 128 * kt:128 * kt + 128],
                    rhs=QT[0][32 * h:32 * h + 32, ts0 + c0:ts0 + TCH],
                    start=True, stop=False, tile_position=(32 * h, 0),
                    skip_group_check=True)
            for h in range(HPC):  # odds wave, accumulate
                nc.tensor.matmul(
                    sc[h][:, c0:TCH],
                    lhsT=KT[1][32 * h:32 * h + 32, 128 * kt:128 * kt + 128],
                    rhs=QT[1][32 * h:32 * h + 32, ts0 + c0:ts0 + TCH],
                    start=False, stop=True, tile_position=(32 * h, 0),
                    skip_group_check=True)
            for h in range(HPC):
                pt = prb.tile([128, 512], MM_DT, tag="pt", name=f"pt_{t}_{kt}_{h}")
                nc.scalar.activation(pt[:, c0:TCH], sc[h][:, c0:TCH],
                                     AF.Exp, scale=0.125)
                if i >= 0:
                    nc.gpsimd.tensor_mul(pt[:, c0:c0 + 128],
                                         pt[:, c0:c0 + 128], mask_sb[:])
                nc.tensor.matmul(
                    att[h][0:65, c0:TCH],
                    lhsT=v_view[:, kt, h, :],
                    rhs=pt[:, c0:TCH],
                    start=(kt == 0), stop=(kt == nkt_t - 1),
                    skip_group_check=True)
        # ---- normalize rows 0:64 by row 64, write into attnT
        for h in range(HPC):
            pr, h2 = divmod(h, 2)
            dr = nrm.tile([1, 512], F32, tag="dr", name=f"dr_{t}_{h}")
            nc.vector.tensor_copy(dr[:], att[h][64:65, :])
            rr = nrm.tile([1, 512], F32, tag="dr", name=f"rr_{t}_{h}")
            nc.vector.reciprocal_approx_fast(rr[:], dr[:])
            rec = nrm.tile([64, 512], F32, tag="den", name=f"rec_{t}_{h}")
            nc.gpsimd.partition_broadcast(rec[:], rr[:])
            if h2 == 0:
                nc.vector.tensor_mul(attnT[pr][0:64, ts0:ts0 + TCH],
                                     att[h][0:64, :], rec[:])
            else:
                tmp = nrm.tile([64, 512], MM_DT, tag="den", name=f"tmp_{t}_{h}")
                nc.vector.tensor_mul(tmp[:], att[h][0:64, :], rec[:])
                nc.vector.tensor_copy(attnT[pr][64:128, ts0:ts0 + TCH], tmp[:])

        # ---- output projection for this token range
        for tt in range(4):
            tg = 4 * t + tt
            o_sb = outsb.tile([128, 1024], F32, tag="osb", name=f"osb_{t}_{tt}")
            for d2 in range(2):
                o_ps = ps.tile([128, 512], F32, tag="sc", bufs=4,
                               name=f"ops_{t}_{tt}_{d2}")
                for r in range(2):
                    nc.tensor.matmul(
                        o_ps[:],
                        lhsT=attnT[r][:, 128 * tg:128 * tg + 128],
                        rhs=wo_sb[:, 1024 * r + 512 * d2:1024 * r + 512 * d2 + 512],
                        start=(r == 0), stop=(r == 1))
                nc.vector.tensor_copy(o_sb[:, 512 * d2:512 * d2 + 512], o_ps[:])
            nc.sync.dma_start(out[128 * tg:128 * tg + 128, :], o_sb[:])


def build_nc(repeat=1):
    nc = bacc.Bacc("TRN2", target_bir_lowering=False, debug=False,
                   enable_asserts=False, num_devices=N_CORES)
    aps = [
        nc.dram_tensor("xT", [DM, SEQ], MM_DT, kind="ExternalInput").ap(),
        nc.dram_tensor("wq", [DM, 256], MM_DT, kind="ExternalInput").ap(),
        nc.dram_tensor("wk", [DM, 256], MM_DT, kind="ExternalInput").ap(),
        nc.dram_tensor("wv", [DM, 256], MM_DT, kind="ExternalInput").ap(),
        nc.dram_tensor("wo", [256, DM], MM_DT, kind="ExternalInput").ap(),
        nc.dram_tensor("cosT", [128, SEQ], MM_DT if MM_DT == BF16 else F32, kind="ExternalInput").ap(),
        nc.dram_tensor("sinT", [128, SEQ], MM_DT if MM_DT == BF16 else F32, kind="ExternalInput").ap(),
        nc.dram_tensor("mask01", [128, 128], MM_DT, kind="ExternalInput").ap(),
        nc.dram_tensor("out", [SEQ, DM], F32, kind="ExternalOutput").ap(),
    ]
    with tile.TileContext(nc) as t:
        _mha_body(t, aps, repeat=repeat)
    nc.compile()
    return nc


_NC = {}


def _get_nc(repeat=1):
    if repeat not in _NC:
        _NC[repeat] = build_nc(repeat)
    return _NC[repeat]


def _qk_perm():
    """Column permutation for one 256-row W shard: cols 0:128 = evens
    (col 32h+i <- shard row 64h+2i), cols 128:256 = odds."""
    perm = []
    for par in range(2):  # 0 = evens, 1 = odds
        for h in range(HPC):
            for i in range(32):
                perm.append(64 * h + 2 * i + par)
    return np.array(perm, dtype=np.int64)


def make_in_maps(x, token_positions, WQ, WK, WV, WO):
    np_mm = mybir.dt.np(MM_DT)
    x = np.asarray(x, dtype=np.float32)
    WQ, WK, WV, WO = (np.asarray(w, dtype=np.float32) for w in (WQ, WK, WV, WO))
    pos = np.asarray(token_positions).astype(np.float64)

    half = DK // 2
    inv = 1.0 / (THETA ** (2.0 * np.arange(half, dtype=np.float64) / DK))
    freqs = pos[:, None] * inv[None, :]                      # [SEQ, 32]
    import ml_dtypes
    _ct = np.float32 if MM_DT == F32R else ml_dtypes.bfloat16
    cosT = np.tile(np.cos(freqs).T, (4, 1)).astype(_ct)  # [128, SEQ]
    sinT = np.tile(np.sin(freqs).T, (4, 1)).astype(_ct)
    cosT = np.ascontiguousarray(cosT)
    sinT = np.ascontiguousarray(sinT)
    mask01 = np.triu(np.ones((128, 128), dtype=np_mm))

    perm = _qk_perm()
    xTs = [np.ascontiguousarray(x[b].T) for b in range(B)]
    in_maps = []
    for c in range(N_CORES):
        b, g = divmod(c, N_CORES // B)
        sl = slice(256 * g, 256 * (g + 1))
        in_maps.append({
            "xT": xTs[b].astype(np_mm),
            "wq": np.ascontiguousarray(WQ[sl, :][perm, :].T).astype(np_mm),
            "wk": np.ascontiguousarray(WK[sl, :][perm, :].T).astype(np_mm),
            "wv": np.ascontiguousarray(WV[sl, :].T).astype(np_mm),
            "wo": np.ascontiguousarray(WO[:, sl].T).astype(np_mm),
            "cosT": cosT,
            "sinT": sinT,
            "mask01": mask01,
        })
    return in_maps


def run(in_maps, trace=False, **kw):
    nc = _get_nc()
    return run_bass_kernel_spmd(nc, in_maps, list(range(N_CORES)), trace=trace, **kw)


def kernel(x, token_positions, WQ, WK, WV, WO):
    in_maps = make_in_maps(x, token_positions, WQ, WK, WV, WO)
    res = run(in_maps)
    out = np.zeros((B, SEQ, DM), dtype=np.float32)
    for c in range(N_CORES):
        out[c // (N_CORES // B)] += res.results[c]["out"]
    return out



# revision 52
# speedup vs baseline: 6.9893x; 6.9893x over previous
"""Multi-head self-attention (RoPE, causal) Trainium2 Bass kernel.

Full inputs in, full output out. Internally shards across 8 NeuronCores:
data-parallel over batch (2) x tensor-parallel over heads (16 -> 4 per core).
Each core computes its 4 heads' attention and the partial WO contraction;
the host sums the 4 partials per batch (the "all-reduce" of the sharding
hint, done on host since outputs are gathered anyway).

v2 design (per core, heads h = 0..3 local):
  xT    [1024, 2048]  x[b].T
  wq/wk [1024, 256]   W shard transposed, columns permuted: cols 0:128 =
                      "evens" (col 32h+i <-> head h, dim 2i), cols 128:256 =
                      "odds". Projection produces QT_E/QT_O [128, 512] PSUM
                      tiles; RoPE is 6 DVE ops reading PSUM directly, writing
                      E/O SBUF tiles; a DMA shuffle then builds merged tiles
                      QTM/KTM [128, 2048] with rows 64h+[0:32]=E, [32:64]=O
                      so scores need ONE 64-contraction matmul per head.
  attention: 256-query chunks, 4 heads side by side in one [128, 1024] PSUM
             group (2 banks); exp is a single wide ACT op per key tile
             (strided AP on diagonal tiles); 0/1 triangular mask applied on
             gpsimd after exp; MM2 accumulates into [65, 1024] PSUM (2 banks)
             via V with a trailing ones column (row 64 = softmax denominator).
  PSUM banks: sc 2x[128,1024] (4) + att [65,1024] (2) + proj [128,512] x2 (2).
  Matmul operands bf16 (fp32 PSUM accum; rel err ~5e-3).
"""

from collections import deque
from contextlib import ExitStack

import numpy as np

import concourse.bass as bass
import concourse.tile as tile
from concourse import bacc, library_config, mybir
from concourse.bass_utils import run_bass_kernel_spmd
from concourse._compat import with_exitstack

F32 = mybir.dt.float32
BF16 = mybir.dt.bfloat16
MM_DT = BF16
AF = mybir.ActivationFunctionType
ALU = mybir.AluOpType

import os as _os
_DEBUG = _os.environ.get("MHA_DEBUG") == "1"
_NO_ILV = _os.environ.get("MHA_NO_ILV") == "1"
_NO_LAG = _os.environ.get("MHA_NO_LAG") == "1"
_NO_DEFER = _os.environ.get("MHA_NO_DEFER") == "1"
_TRUNC = _os.environ.get("MHA_TRUNC") == "1"
_PHASE = int(_os.environ.get("MHA_PHASE", "3"))
_CAPKT = int(_os.environ.get("MHA_CAPKT", "0"))
_ATTB = int(_os.environ.get("MHA_ATTB", "1"))
_SCB = int(_os.environ.get("MHA_SCB", "2"))

B = 2
SEQ = 2048
DM = 1024
H = 16
DK = 64
THETA = 10000.0

N_CORES = 8
HPC = 4           # heads per core
PCH = 512         # projection token chunk
NPC = SEQ // PCH  # 4
ACH = 256         # attention query chunk
NAC = SEQ // ACH  # 8
KD = DM // 128    # 8 contraction chunks
NKT = SEQ // 128  # 16 key tiles


@with_exitstack
def _mha_body(ctx: ExitStack, tc_: tile.TileContext, aps, repeat=1):
    nc = tc_.nc
    xT, wq, wk, wv, wo, cosT, sinT, mask4, out = aps

    const = ctx.enter_context(tc_.tile_pool(name="const", bufs=1))
    xkp = ctx.enter_context(tc_.tile_pool(name="xkp", bufs=2))
    work = ctx.enter_context(tc_.tile_pool(name="work", bufs=4))
    prb = ctx.enter_context(tc_.tile_pool(name="prb", bufs=4))
    nrm = ctx.enter_context(tc_.tile_pool(name="nrm", bufs=4))
    outsb = ctx.enter_context(tc_.tile_pool(name="outsb", bufs=3))
    ps = ctx.enter_context(tc_.tile_pool(name="ps", bufs=1, space="PSUM"))

    nc.gpsimd.load_library(library_config.proxy)

    # ---- constants / weights to SBUF (xk0 is DMA'd first, see pipeline)
    wq_sb = const.tile([128, 2048], MM_DT, name="wq_sb")
    wk_sb = const.tile([128, 2048], MM_DT, name="wk_sb")
    wv_sb = const.tile([128, 2048], MM_DT, name="wv_sb")
    wo_sb = const.tile([128, 2048], MM_DT, name="wo_sb")
    cos_sb = const.tile([128, 2048], MM_DT, name="cos_sb")
    sin_sb = const.tile([128, 2048], MM_DT, name="sin_sb")
    mask_sb = const.tile([128, 512], MM_DT, name="mask_sb")

    def emit_const_loads():
        nc.sync.dma_start(wv_sb.rearrange("p (k m) -> p k m", k=KD),
                          wv.rearrange("(k p) m -> p k m", p=128))
        nc.sync.dma_start(wq_sb.rearrange("p (k m) -> p k m", k=KD),
                          wq.rearrange("(k p) m -> p k m", p=128))
        nc.sync.dma_start(wk_sb.rearrange("p (k m) -> p k m", k=KD),
                          wk.rearrange("(k p) m -> p k m", p=128))
        nc.sync.dma_start(cos_sb[:], cosT[:])
        nc.sync.dma_start(sin_sb[:], sinT[:])
        nc.sync.dma_start(mask_sb[:], mask4[:])
        nc.sync.dma_start(wo_sb.rearrange("p (r d) -> p r d", r=2),
                          wo.rearrange("(r p) d -> p r d", p=128))

    # persistent per-core state
    QT = [const.tile([128, 2048], MM_DT, name=f"QT{m}") for m in range(2)]
    KT = [const.tile([128, 2048], MM_DT, name=f"KT{m}") for m in range(2)]
    # merged layout: tile m holds heads {2m,2m+1}; rows 64h'+[0:32]=E, [32:64]=O
    QTM = [const.tile([128, 2048], MM_DT, name=f"QTM{m}") for m in range(2)]
    KTM = [const.tile([128, 2048], MM_DT, name=f"KTM{m}") for m in range(2)]
    attnT = [const.tile([128, 2048], MM_DT, name=f"attnT{m}") for m in range(2)]
    V_sb = const.tile([128, NKT * HPC * 65], MM_DT, name="V_sb")
    v_view = V_sb.rearrange("p (g h m) -> p g h m", h=HPC, m=65)
    # softmax-denominator ones columns, via ACT copy (memset can't write bf16?)
    nc.scalar.activation(v_view[:, :, :, 64],
                         mask_sb[:, 0:64].rearrange("p (g h) -> p g h", h=HPC),
                         AF.Copy, bias=1.0, scale=0.0)

    pending = deque()
    pending2 = deque()  # low-priority (out-proj): drained when pending empty

    def pull(n):
        for _ in range(n):
            if pending:
                pending.popleft()()
            elif pending2:
                pending2.popleft()()

    def emit_xload(pt):
        ts0 = PCH * pt
        xk_t = xkp.tile([128, KD * PCH], MM_DT, name=f"xk_{pt}", tag="xk",
                        bufs=2)
        nc.sync.dma_start(
            xk_t.rearrange("p (k c) -> p k c", k=KD),
            xT[:, ts0:ts0 + PCH].rearrange("(k p) c -> p k c", p=128))
        return xk_t

    def proj_ops(pt, xk_t):
        """Closures for V/Q/K projection + RoPE + merge-shuffle of chunk pt."""
        ts0 = PCH * pt
        xks = [xk_t[:, PCH * k:PCH * k + PCH] for k in range(KD)]
        ops = []

        # V projection (each psum tile covers 2 key-tiles)
        for vp in range(2):
            v_ps = ps.tile([128, 512], F32, tag="proj", bufs=2,
                           name=f"vps_{pt}_{vp}")
            for half in range(2):
                lkt = 2 * vp + half
                for k in range(KD):
                    def mm(v_ps=v_ps, half=half, lkt=lkt, k=k):
                        nc.tensor.matmul(
                            v_ps[:, 256 * half:256 * half + 256],
                            lhsT=xks[k][:, 128 * lkt:128 * lkt + 128],
                            rhs=wv_sb[:, 256 * k:256 * k + 256],
                            start=(k == 0), stop=(k == KD - 1),
                            skip_group_check=True)
                    ops.append(mm)
            gkt = 4 * pt + 2 * vp
            def vdrain(v_ps=v_ps, gkt=gkt):
                nc.vector.tensor_copy(
                    v_view[:, gkt:gkt + 2, :, 0:64],
                    v_ps.rearrange("p (x h m) -> p x h m", x=2, h=HPC))
            ops.append(vdrain)

        # Q/K projections (E then O) + fused RoPE (reads PSUM) + merge DMA
        for w_sb, dstT, dstM, wtag in ((wq_sb, QT, QTM, "q"),
                                       (wk_sb, KT, KTM, "k")):
            res = [None, None]  # (t_cos, t_sin) per parity, shared w/ closures
            for m in range(2):
                q_ps = ps.tile([128, 512], F32, tag="proj", bufs=2,
                               name=f"qps_{pt}_{m}_{wtag}")
                for k in range(KD):
                    def mm(q_ps=q_ps, m=m, k=k, w_sb=w_sb):
                        nc.tensor.matmul(
                            q_ps[:],
                            lhsT=w_sb[:, 256 * k + 128 * m:256 * k + 128 * m + 128],
                            rhs=xks[k][:],
                            start=(k == 0), stop=(k == KD - 1))
                    ops.append(mm)
                # RoPE partial products for this parity, fused from PSUM
                def ropemul(q_ps=q_ps, m=m, wtag=wtag, res=res):
                    cs = cos_sb[:, ts0:ts0 + PCH]
                    sn = sin_sb[:, ts0:ts0 + PCH]
                    tc1 = work.tile([128, 512], MM_DT, tag="ro",
                                    name=f"tc_{pt}_{m}_{wtag}")
                    nc.vector.tensor_mul(tc1[:], q_ps[:], cs)
                    tsn = work.tile([128, 512], MM_DT, tag="ro",
                                    name=f"ts_{pt}_{m}_{wtag}")
                    nc.vector.tensor_mul(tsn[:], q_ps[:], sn)
                    res[m] = (tc1, tsn)
                ops.append(ropemul)

            def ropefin(dstT=dstT, res=res):
                (tcE, tsE), (tcO, tsO) = res
                # E' = E*cos - O*sin ; O' = O*cos + E*sin
                nc.vector.scalar_tensor_tensor(
                    dstT[0][:, ts0:ts0 + PCH], tsO[:], -1.0, tcE[:],
                    op0=ALU.mult, op1=ALU.add)
                nc.vector.tensor_add(dstT[1][:, ts0:ts0 + PCH], tcO[:], tsE[:])
            ops.append(ropefin)

            def shuffle(dstT=dstT, dstM=dstM):
                # merged tile m rows: [h'E 0:32 | h'O 32:64 | h''E 64:96 |
                # h''O 96:128]; SBUF APs allow only one contiguous partition
                # range, so 4 DMAs per tile
                for m in range(2):
                    for hh in range(2):
                        for eo in range(2):
                            nc.sync.dma_start(
                                dstM[m][64 * hh + 32 * eo:
                                        64 * hh + 32 * eo + 32,
                                        ts0:ts0 + PCH],
                                dstT[eo][64 * m + 32 * hh:
                                         64 * m + 32 * hh + 32,
                                         ts0:ts0 + PCH])
            ops.append(shuffle)

        return ops

    def outproj_ops(ac):
        """WO contraction + drain + store for attention chunk ac."""
        ops = []
        for tt in range(2):
            tg = 2 * ac + tt
            def op_one(tg=tg):
                o_sb = outsb.tile([128, 1024], F32, tag="osb",
                                  name=f"osb_{tg}")
                for d2 in range(2):
                    o_ps = ps.tile([128, 512], F32, tag="proj", bufs=2,
                                   name=f"ops_{tg}_{d2}")
                    for r in range(2):
                        nc.tensor.matmul(
                            o_ps[:],
                            lhsT=attnT[r][:, 128 * tg:128 * tg + 128],
                            rhs=wo_sb[:, 1024 * r + 512 * d2:
                                      1024 * r + 512 * d2 + 512],
                            start=(r == 0), stop=(r == 1))
                    nc.vector.tensor_copy(o_sb[:, 512 * d2:512 * d2 + 512],
                                          o_ps[:])
                nc.sync.dma_start(out[128 * tg:128 * tg + 128, :], o_sb[:])
            ops.append(op_one)
        return ops

    def attention(ac):
        """Causal attention for query chunk ac (256 queries, 4 heads wide)."""
        q0 = ACH * ac
        nkt = 2 * ac + 2
        if _CAPKT:
            nkt = min(nkt, _CAPKT)
        per = max(2, (len(pending) + nkt - 1) // nkt)
        att = ps.tile([128, 1024], F32, tag="att", bufs=_ATTB,
                      name=f"att_{ac}")
        sc_tiles = {}
        pt_tiles = {}

        # head h -> column base within the 4-head-wide tiles. Heads at
        # tile_position 0 vs 64 run CONCURRENTLY on the PE (row-group
        # tiling), so they must write different PSUM banks: bank0 (cols
        # 0:512) holds heads {0,2}, bank1 holds {1,3}.
        def hcol(h):
            return 512 * (h % 2) + 256 * (h // 2)

        def emit_sc(kt, first=False):
            i = kt - 2 * ac
            c0 = 128 if i == 1 else 0
            sc = ps.tile([128, 1024], F32, tag="sc", bufs=_SCB,
                         name=f"sc_{ac}_{kt}")
            sc_tiles[kt] = (sc, c0)
            # PSUM start marks the whole 2KB bank pending-zero, so only the
            # first head of each bank-pair may carry start=True
            for h in range(HPC):
                m, hh = divmod(h, 2)
                nc.tensor.matmul(
                    sc[:, hcol(h) + c0:hcol(h) + 256],
                    lhsT=KTM[m][64 * hh:64 * hh + 64,
                                128 * kt:128 * kt + 128],
                    rhs=QTM[m][64 * hh:64 * hh + 64, q0 + c0:q0 + ACH],
                    start=(h < 2), stop=(h >= 2),
                    tile_position=(64 * hh, 0),
                    skip_group_check=True)

        def emit_exp_mask(kt):
            sc, c0 = sc_tiles[kt]
            i = kt - 2 * ac
            pt_t = prb.tile([128, 1024], MM_DT, tag="pt", bufs=6,
                            name=f"pt_{ac}_{kt}")
            pt_tiles[kt] = (pt_t, c0)
            if c0 == 0:
                nc.scalar.activation(pt_t[:], sc[:], AF.Exp, scale=0.125)
            else:
                # contiguous 2D slices only (strided 3D APs break on HW)
                for h in range(HPC):
                    nc.scalar.activation(
                        pt_t[:, hcol(h) + c0:hcol(h) + 256],
                        sc[:, hcol(h) + c0:hcol(h) + 256],
                        AF.Exp, scale=0.125)
                    # zero the unwritten [0:c0] slice so MM2 can accumulate
                    # the full 256-col range (PSUM start regions are per-bank)
                    nc.gpsimd.memset(pt_t[:, hcol(h):hcol(h) + c0], 0.0)
            if i >= 0:  # triangular 0/1 mask on the 128-col diagonal block
                for h in range(HPC):
                    ptm = pt_t[:, hcol(h) + c0:hcol(h) + c0 + 128]
                    nc.gpsimd.tensor_mul(ptm, ptm, mask_sb[:, 0:128])
            if _DEBUG and ac == 0:
                d2 = const.tile([128, 1024], MM_DT, name=f"dbg_pt{kt}")
                nc.vector.tensor_copy(d2[:], pt_t[:])

        def emit_mm2(kt, is_first, is_last):
            pt_t, c0 = pt_tiles.pop(kt)
            for h in range(HPC):
                nc.tensor.matmul(
                    att[0:65, hcol(h):hcol(h) + 256],
                    lhsT=v_view[:, kt, h, :],
                    rhs=pt_t[:, hcol(h):hcol(h) + 256],
                    start=(is_first and h < 2),
                    stop=(is_last and h >= 2),
                    skip_group_check=True)

        LAG = 0 if _NO_LAG else 2
        # diagonal key-tiles first: their Pool-mask dependency then has the
        # whole loop as cushion instead of stalling the tail MM2s
        if _NO_LAG:
            order = list(range(nkt))
        else:
            order = [nkt - 2, nkt - 1] + list(range(nkt - 2))
        for step, kt in enumerate(order):
            emit_sc(kt, first=(step == 0))
            emit_exp_mask(kt)
            pull(per)
            if step >= LAG:
                emit_mm2(order[step - LAG], step - LAG == 0,
                         step - LAG == nkt - 1)
        for step in range(max(0, nkt - LAG), nkt):
            emit_mm2(order[step], step == 0, step == nkt - 1)

        # drain att psum fast (frees the banks), defer normalize to the
        # pull stream so the next chunk's MM2 isn't blocked on it
        araw = nrm.tile([128, 1024], F32, tag="araw", bufs=2,
                        name=f"araw_{ac}")
        nc.vector.tensor_copy(araw[0:65, :], att[0:65, :])
        if _DEBUG and ac == 0:
            d = const.tile([128, 1024], F32, name="dbg_att0")
            nc.vector.tensor_copy(d[0:65, :], att[0:65, :])

        def normalize(araw=araw, q0=q0, ac=ac):
            # copy denom row to a partition-0 tile first: the custom-DVE
            # reciprocal ucode needs a partition-0-based input on HW
            dr = nrm.tile([1, 1024], F32, tag="rr", name=f"dr_{ac}")
            nc.vector.tensor_copy(dr[:], araw[64:65, :])
            rr = nrm.tile([1, 1024], F32, tag="rr", name=f"rr_{ac}")
            nc.vector.reciprocal_approx_fast(rr[:], dr[:])
            rec = nrm.tile([64, 1024], F32, tag="rec", bufs=2,
                           name=f"rec_{ac}")
            nc.gpsimd.partition_broadcast(rec[:], rr[:])
            if _DEBUG and ac == 0:
                da = const.tile([128, 1024], F32, name="dbg_araw0")
                nc.vector.tensor_copy(da[0:65, :], araw[0:65, :])
                dr = const.tile([1, 1024], F32, name="dbg_rr0")
                nc.vector.tensor_copy(dr[:], rr[:])
                dc = const.tile([64, 1024], F32, name="dbg_rec0")
                nc.vector.tensor_copy(dc[:], rec[:])
            for h in range(HPC):
                pr, h2 = divmod(h, 2)
                hc = 512 * (h % 2) + 256 * (h // 2)
                if h2 == 0:
                    nc.vector.tensor_mul(attnT[pr][0:64, q0:q0 + ACH],
                                         araw[0:64, hc:hc + 256],
                                         rec[:, hc:hc + 256])
                else:
                    tmp = nrm.tile([64, 256], MM_DT, tag="tmp",
                                   name=f"tmp_{ac}_{h}")
                    nc.vector.tensor_mul(tmp[:],
                                         araw[0:64, hc:hc + 256],
                                         rec[:, hc:hc + 256])
                    nc.vector.tensor_copy(attnT[pr][64:128, q0:q0 + ACH],
                                          tmp[:])
        if _NO_DEFER:
            normalize()
        else:
            pending.append(normalize)

    # ---- software pipeline: proj(pt) interleaved into attention of pt-1
    loop_cm = tc_.For_i(0, repeat) if repeat > 1 else None
    if loop_cm is not None:
        ctx.enter_context(loop_cm)

    xk0 = emit_xload(0)
    emit_const_loads()
    xk_next = [emit_xload(1)]
    if _NO_ILV:
        for pt in range(1 if _TRUNC else NPC):
            xk_t = xk0 if pt == 0 else xk_next[0]
            if pt > 0 and pt + 1 <= NPC - 1:
                xk_next[0] = emit_xload(pt + 1)
            for op in proj_ops(pt, xk_t):
                op()
            if _PHASE >= 1:
                attention(2 * pt)
                pull(len(pending))
            if _PHASE >= 2:
                attention(2 * pt + 1)
                pull(len(pending))
            if _PHASE >= 3:
                for op in outproj_ops(2 * pt) + outproj_ops(2 * pt + 1):
                    op()
        return
    for op in proj_ops(0, xk0):
        op()
    for pt in range(1, NPC + 1):
        if pt <= NPC - 1:
            xk_t = xk_next[0]
            if pt + 1 <= NPC - 1:
                xk_next[0] = emit_xload(pt + 1)
            pending.extend(proj_ops(pt, xk_t))
        attention(2 * pt - 2)
        attention(2 * pt - 1)
        pending2.extend(outproj_ops(2 * pt - 2))
        pending2.extend(outproj_ops(2 * pt - 1))
    pull(len(pending) + len(pending2))


def build_nc(repeat=1):
    nc = bacc.Bacc("TRN2", target_bir_lowering=False, debug=False,
                   enable_asserts=False, num_devices=N_CORES)
    aps = [
        nc.dram_tensor("xT", [DM, SEQ], MM_DT, kind="ExternalInput").ap(),
        nc.dram_tensor("wq", [DM, 256], MM_DT, kind="ExternalInput").ap(),
        nc.dram_tensor("wk", [DM, 256], MM_DT, kind="ExternalInput").ap(),
        nc.dram_tensor("wv", [DM, 256], MM_DT, kind="ExternalInput").ap(),
        nc.dram_tensor("wo", [256, DM], MM_DT, kind="ExternalInput").ap(),
        nc.dram_tensor("cosT", [128, SEQ], MM_DT, kind="ExternalInput").ap(),
        nc.dram_tensor("sinT", [128, SEQ], MM_DT, kind="ExternalInput").ap(),
        nc.dram_tensor("mask4", [128, 512], MM_DT, kind="ExternalInput").ap(),
        nc.dram_tensor("out", [SEQ, DM], F32, kind="ExternalOutput").ap(),
    ]
    with tile.TileContext(nc) as t:
        _mha_body(t, aps, repeat=repeat)
    nc.compile()
    return nc


_NC = {}


def _get_nc(repeat=1):
    if repeat not in _NC:
        _NC[repeat] = build_nc(repeat)
    return _NC[repeat]


def _qk_perm():
    """Column permutation for one 256-row W shard: cols 0:128 = evens
    (col 32h+i <- shard row 64h+2i), cols 128:256 = odds."""
    perm = []
    for par in range(2):  # 0 = evens, 1 = odds
        for h in range(HPC):
            for i in range(32):
                perm.append(64 * h + 2 * i + par)
    return np.array(perm, dtype=np.int64)


def make_in_maps(x, token_positions, WQ, WK, WV, WO):
    np_mm = mybir.dt.np(MM_DT)
    x = np.asarray(x, dtype=np.float32)
    WQ, WK, WV, WO = (np.asarray(w, dtype=np.float32) for w in (WQ, WK, WV, WO))
    pos = np.asarray(token_positions).astype(np.float64)

    half = DK // 2
    inv = 1.0 / (THETA ** (2.0 * np.arange(half, dtype=np.float64) / DK))
    freqs = pos[:, None] * inv[None, :]                      # [SEQ, 32]
    import ml_dtypes
    _ct = ml_dtypes.bfloat16
    cosT = np.ascontiguousarray(np.tile(np.cos(freqs).T, (4, 1)).astype(_ct))
    sinT = np.ascontiguousarray(np.tile(np.sin(freqs).T, (4, 1)).astype(_ct))
    mask4 = np.ascontiguousarray(
        np.tile(np.triu(np.ones((128, 128), dtype=np_mm)), (1, 4)))

    perm = _qk_perm()
    xTs = [np.ascontiguousarray(x[b].T) for b in range(B)]
    in_maps = []
    for c in range(N_CORES):
        b, g = divmod(c, N_CORES // B)
        sl = slice(256 * g, 256 * (g + 1))
        in_maps.append({
            "xT": xTs[b].astype(np_mm),
            "wq": np.ascontiguousarray(WQ[sl, :][perm, :].T).astype(np_mm),
            "wk": np.ascontiguousarray(WK[sl, :][perm, :].T).astype(np_mm),
            "wv": np.ascontiguousarray(WV[sl, :].T).astype(np_mm),
            "wo": np.ascontiguousarray(WO[:, sl].T).astype(np_mm),
            "cosT": cosT,
            "sinT": sinT,
            "mask4": mask4,
        })
    return in_maps


def run(in_maps, trace=False, **kw):
    nc = _get_nc()
    return run_bass_kernel_spmd(nc, in_maps, list(range(N_CORES)), trace=trace, **kw)


def kernel(x, token_positions, WQ, WK, WV, WO):
    in_maps = make_in_maps(x, token_positions, WQ, WK, WV, WO)
    res = run(in_maps)
    out = np.zeros((B, SEQ, DM), dtype=np.float32)
    for c in range(N_CORES):
        out[c // (N_CORES // B)] += res.results[c]["out"]
    return out
